# revision 1
# baseline (speedup 1.0000x reference)
"""Trainium2 Bass kernel for nn_CrossAttention (B=4, L=2048, D=1024, H=8).

Sharding: 8 cores = 4 batches x 2 query-halves (data parallel over B x Lq).
Each core computes, for its (b, half):
    Qn = LN(Q_slice); Kn = LN(K_b)            (pre_g folded into Wq/Wk rows,
                                               pre_b -> bias row on q/k)
    q = Qn @ Wq ; k = Kn @ Wk ; v = V_b @ Wv   (fp32r matmuls, feature-major)
    S_h^T = k_h q_h^T / TEMP  -> exp on ScalarE -> sums via ones-matmul on PE
    O^T_h = accumulation of v_h^T exp(S^T) on PE (unnormalized)
    O = LN(transpose(O^T) / sums) ; out = O + gelu(O @ Wo)

All heavy matmuls run in float32r (fp32 with 12-bit-truncated mantissa,
pre-rounded on host or rounded on-chip at PSUM evictions), which streams at
bf16 rate for free-dim >= 256.
"""

import numpy as np

P = 128
D = 1024
H = 8
HD = 128
LQ = 1024  # per-core query rows
LK = 2048
N_CORES = 8
TEMP = 32.0  # sqrt(D)
EPS = 1e-5

_PROGRAM_CACHE = {}


def round_fp32r(x: np.ndarray) -> np.ndarray:
    """Round fp32 to fp32r (12 low mantissa bits, round-to-nearest-even)."""
    u = np.ascontiguousarray(x).view(np.uint32)
    low = u & np.uint32(0xFFF)
    half = np.uint32(0x800)
    base = u & np.uint32(0xFFFFF000)
    rnd = np.where(
        (low > half)
        | ((low == half) & ((u >> np.uint32(12)) & np.uint32(1)).astype(bool)),
        base + np.uint32(0x1000),
        base,
    )
    return rnd.astype(np.uint32).view(np.float32)


def _build_program(has_qk_bias: bool, trivial_ln: bool):
    import concourse.bacc as bacc
    import concourse.mybir as mybir
    import concourse.tile as tile
    from contextlib import ExitStack

    FP32 = mybir.dt.float32
    FP32R = mybir.dt.float32r
    AF = mybir.ActivationFunctionType
    SUB = mybir.AluOpType.subtract
    MULT = mybir.AluOpType.mult

    nc = bacc.Bacc("TRN2", target_bir_lowering=False, debug=False)

    # ---- DRAM I/O ----
    Qs = nc.dram_tensor("Qs", [LQ, D], FP32, kind="ExternalInput")
    Kf = nc.dram_tensor("Kf", [LK, D], FP32, kind="ExternalInput")
    Vf = nc.dram_tensor("Vf", [LK, D], FP32, kind="ExternalInput")
    Wq_d = nc.dram_tensor("Wq_r", [D, D], FP32R, kind="ExternalInput")
    Wk_d = nc.dram_tensor("Wk_r", [D, D], FP32R, kind="ExternalInput")
    Wv_d = nc.dram_tensor("Wv_r", [D, D], FP32R, kind="ExternalInput")
    Wo_d = nc.dram_tensor("Wo_r", [D, D], FP32R, kind="ExternalInput")
    ID_R = nc.dram_tensor("ID_R", [P, P], FP32R, kind="ExternalInput")
    ID_F = nc.dram_tensor("ID_F", [P, P], FP32, kind="ExternalInput")
    ONES_D = nc.dram_tensor("ONES", [P, 1], FP32R, kind="ExternalInput")
    if has_qk_bias:
        BQ_D = nc.dram_tensor("BQ", [P, H], FP32, kind="ExternalInput")
        BK_D = nc.dram_tensor("BK", [P, H], FP32, kind="ExternalInput")
    if not trivial_ln:
        LNG_D = nc.dram_tensor("LNG_B", [P, D], FP32, kind="ExternalInput")
        LNB_D = nc.dram_tensor("LNB_B", [P, D], FP32, kind="ExternalInput")
    OUT = nc.dram_tensor("OUT", [LQ, D], FP32, kind="ExternalOutput")

    with tile.TileContext(nc) as tc, ExitStack() as top:
        singles = top.enter_context(tc.tile_pool(name="singles", bufs=1))
        dram_pool = top.enter_context(tc.tile_pool(name="dram", bufs=1, space="DRAM"))
        ident_r = singles.tile([P, P], FP32R)
        ident_f = singles.tile([P, P], FP32)
        ones = singles.tile([P, 1], FP32R)
        eps_t = singles.tile([P, 1], FP32)
        nc.sync.dma_start(ident_r[:], ID_R.ap())
        nc.sync.dma_start(ident_f[:], ID_F.ap())
        nc.sync.dma_start(ones[:], ONES_D.ap())
        nc.vector.memset(eps_t[:], EPS)
        if has_qk_bias:
            bq_sb = singles.tile([P, H], FP32)
            bk_sb = singles.tile([P, H], FP32)
            nc.sync.dma_start(bq_sb[:], BQ_D.ap())
            nc.sync.dma_start(bk_sb[:], BK_D.ap())
        sums_dram = dram_pool.tile([H, LQ], FP32R)
        ot_dram = dram_pool.tile([H, P, LQ], FP32R)

        def layernorm(pool, x_tile, out_tile):
            """LN over free dim (D=1024) of [128, D] fp32 tile -> fp32r out."""
            xr = x_tile[:].rearrange("p (n f) -> p n f", f=512)
            stats = pool.tile([P, 2, nc.vector.BN_STATS_DIM], FP32, tag="ln_stats")
            for i in range(2):
                nc.vector.bn_stats(out=stats[:, i, :], in_=xr[:, i, :])
            mv = pool.tile([P, nc.vector.BN_AGGR_DIM], FP32, tag="ln_mv")
            nc.vector.bn_aggr(out=mv[:], in_=stats[:])
            rstd = pool.tile([P, 1], FP32, tag="ln_rstd")
            nc.scalar.activation(
                out=rstd[:], in_=mv[:, 1:2], func=AF.Sqrt, bias=eps_t[:]
            )
            nc.vector.reciprocal(out=rstd[:], in_=rstd[:])
            nc.vector.tensor_scalar(
                out=out_tile[:],
                in0=x_tile[:],
                scalar1=mv[:, 0:1],
                scalar2=rstd[:],
                op0=SUB,
                op1=MULT,
            )

        act_store = ExitStack()
        kth_pool = act_store.enter_context(tc.tile_pool(name="kth", bufs=8))
        qth_pool = act_store.enter_context(tc.tile_pool(name="qth", bufs=8))
        kth = [
            kth_pool.tile([P, LK], FP32R, tag="kth", name=f"kth{h}")
            for h in range(H)
        ]
        qth = [
            qth_pool.tile([P, LQ], FP32R, tag="qth", name=f"qth{h}")
            for h in range(H)
        ]

        # ============ P1/P2: LN + transpose + q/k projections ============
        with ExitStack() as kq_ctx:
            wpool = kq_ctx.enter_context(tc.tile_pool(name="wpool", bufs=8))
            xload = kq_ctx.enter_context(tc.tile_pool(name="xload", bufs=3))
            lnx = kq_ctx.enter_context(tc.tile_pool(name="lnx", bufs=2))
            xtj = kq_ctx.enter_context(tc.tile_pool(name="xtj", bufs=16))
            tp_ps = kq_ctx.enter_context(
                tc.tile_pool(name="tp_ps", bufs=4, space="PSUM")
            )
            pr_ps = kq_ctx.enter_context(
                tc.tile_pool(name="pr_ps", bufs=4, space="PSUM")
            )

            def load_w(w_dram):
                w_sb = []
                for c in range(8):
                    wt = wpool.tile([P, D], FP32R, tag="w")
                    nc.sync.dma_start(wt[:], w_dram.ap()[c * P : (c + 1) * P, :])
                    w_sb.append(wt)
                return w_sb

            def project_T(x_dram, n_tiles, w_sb, out_heads, bias_sb):
                """out_heads[h][:, j] = (LN(X) @ W)^T per head (feature-major)."""
                for J in range(n_tiles // 4):
                    xt_J = [
                        xtj.tile([P, 512], FP32R, tag="xtj", name=f"xtj{c}")
                        for c in range(8)
                    ]
                    for tj in range(4):
                        t = J * 4 + tj
                        xt = xload.tile([P, D], FP32, tag="xload")
                        nc.sync.dma_start(xt[:], x_dram.ap()[t * P : (t + 1) * P, :])
                        xn = lnx.tile([P, D], FP32R, tag="lnx")
                        layernorm(lnx, xt, xn)
                        for c in range(8):
                            tp = tp_ps.tile([P, P], FP32R, tag="tp_r")
                            nc.tensor.transpose(
                                tp[:], xn[:, c * P : (c + 1) * P], ident_r[:]
                            )
                            nc.any.tensor_copy(
                                xt_J[c][:, tj * P : (tj + 1) * P], tp[:]
                            )
                    for h in range(H):
                        ps = pr_ps.tile([P, 512], FP32, tag="pr")
                        for c in range(8):
                            nc.tensor.matmul(
                                ps[:],
                                w_sb[c][:, h * HD : (h + 1) * HD],
                                xt_J[c][:],
                                start=(c == 0),
                                stop=(c == 7),
                            )
                        dst = out_heads[h][:, J * 512 : (J + 1) * 512]
                        if bias_sb is not None:
                            nc.vector.tensor_scalar_add(
                                out=dst, in0=ps[:], scalar1=bias_sb[:, h : h + 1]
                            )
                        else:
                            nc.vector.tensor_copy(dst, ps[:])

            wk_sb = load_w(Wk_d)
            project_T(Kf, 16, wk_sb, kth, bk_sb if has_qk_bias else None)
            wq_sb = load_w(Wq_d)
            project_T(Qs, 8, wq_sb, qth, bq_sb if has_qk_bias else None)

        # ============ P3: v = V @ Wv (row-major out) ============
        v_pool = act_store.enter_context(tc.tile_pool(name="vnat", bufs=16))
        vnat = [
            v_pool.tile([P, D], FP32R, tag="vnat", name=f"vnat{t}")
            for t in range(16)
        ]
        with ExitStack() as v_ctx:
            wpool2 = v_ctx.enter_context(tc.tile_pool(name="wpool2", bufs=8))
            xload2 = v_ctx.enter_context(tc.tile_pool(name="xload2", bufs=2))
            vt_pool = v_ctx.enter_context(tc.tile_pool(name="vt", bufs=8))
            tp_ps2 = v_ctx.enter_context(
                tc.tile_pool(name="tp_ps2", bufs=4, space="PSUM")
            )
            pr_ps2 = v_ctx.enter_context(
                tc.tile_pool(name="pr_ps2", bufs=4, space="PSUM")
            )
            wv_sb = []
            for c in range(8):
                wt = wpool2.tile([P, D], FP32R, tag="wv")
                nc.sync.dma_start(wt[:], Wv_d.ap()[c * P : (c + 1) * P, :])
                wv_sb.append(wt)
            for t in range(16):
                xt = xload2.tile([P, D], FP32, tag="xload2")
                nc.sync.dma_start(xt[:], Vf.ap()[t * P : (t + 1) * P, :])
                vt_blocks = []
                for c in range(8):
                    tp = tp_ps2.tile([P, P], FP32, tag="tp_f")
                    nc.tensor.transpose(
                        tp[:], xt[:, c * P : (c + 1) * P], ident_f[:]
                    )
                    vb = vt_pool.tile([P, P], FP32R, tag="vt")
                    nc.any.tensor_copy(vb[:], tp[:])
                    vt_blocks.append(vb)
                for s in range(2):
                    ps = pr_ps2.tile([P, 512], FP32, tag="pr2")
                    for c in range(8):
                        nc.tensor.matmul(
                            ps[:],
                            vt_blocks[c][:],
                            wv_sb[c][:, s * 512 : (s + 1) * 512],
                            start=(c == 0),
                            stop=(c == 7),
                        )
                    nc.vector.tensor_copy(vnat[t][:, s * 512 : (s + 1) * 512], ps[:])

        # ============ P4: attention per head (feature-major) ============
        with ExitStack() as att_ctx:
            ex_pool = att_ctx.enter_context(tc.tile_pool(name="expst", bufs=4))
            oev_pool = att_ctx.enter_context(tc.tile_pool(name="oev", bufs=2))
            srow_pool = att_ctx.enter_context(tc.tile_pool(name="srow", bufs=1))
            st_ps_pool = att_ctx.enter_context(
                tc.tile_pool(name="st_ps", bufs=4, space="PSUM")
            )
            ot_ps_pool = att_ctx.enter_context(
                tc.tile_pool(name="ot_ps", bufs=1, space="PSUM")
            )
            sm_ps_pool = att_ctx.enter_context(
                tc.tile_pool(name="sm_ps", bufs=1, space="PSUM")
            )
            for h in range(H):
                sums_ps = sm_ps_pool.tile([1, LQ], FP32, tag="sums")
                ot_ps = ot_ps_pool.tile([P, LQ], FP32, tag="otp")
                for jc in range(16):
                    for s in range(2):
                        sl = slice(s * 512, (s + 1) * 512)
                        stp = st_ps_pool.tile([P, 512], FP32, tag="st")
                        nc.tensor.matmul(
                            stp[:],
                            kth[h][:, jc * P : (jc + 1) * P],
                            qth[h][:, sl],
                            start=True,
                            stop=True,
                        )
                        ex = ex_pool.tile([P, 512], FP32R, tag="ex")
                        nc.scalar.activation(ex[:], stp[:], AF.Exp, scale=1.0 / TEMP)
                        nc.tensor.matmul(
                            sums_ps[:, sl],
                            ones[:],
                            ex[:],
                            start=(jc == 0),
                            stop=(jc == 15),
                        )
                        nc.tensor.matmul(
                            ot_ps[:, sl],
                            vnat[jc][:, h * HD : (h + 1) * HD],
                            ex[:],
                            start=(jc == 0),
                            stop=(jc == 15),
                        )
                oev = oev_pool.tile([P, LQ], FP32R, tag="oev")
                nc.vector.tensor_copy(oev[:], ot_ps[:])
                nc.sync.dma_start(ot_dram[h], oev[:])
                srow = srow_pool.tile([1, LQ], FP32R, tag="srow")
                nc.vector.tensor_copy(srow[:], sums_ps[:])
                nc.sync.dma_start(sums_dram[h : h + 1, :], srow[:])

        act_store.close()

        # ======== D: transpose back + 1/sums, LN, Wo, gelu, residual ========
        with ExitStack() as fin_ctx:
            wo_pool = fin_ctx.enter_context(tc.tile_pool(name="wo", bufs=8))
            otl_pool = fin_ctx.enter_context(tc.tile_pool(name="otl", bufs=2))
            onat_pool = fin_ctx.enter_context(tc.tile_pool(name="onat", bufs=8))
            oln_pool = fin_ctx.enter_context(tc.tile_pool(name="oln", bufs=8))
            olnT_pool = fin_ctx.enter_context(tc.tile_pool(name="olnT", bufs=8))
            fsmall = fin_ctx.enter_context(tc.tile_pool(name="fsmall", bufs=3))
            tp2_ps = fin_ctx.enter_context(
                tc.tile_pool(name="tp2_ps", bufs=4, space="PSUM")
            )
            tps_ps = fin_ctx.enter_context(
                tc.tile_pool(name="tps_ps", bufs=1, space="PSUM")
            )
            g_ps_pool = fin_ctx.enter_context(
                tc.tile_pool(name="g_ps", bufs=3, space="PSUM")
            )

            wo_sb = []
            for c in range(8):
                wt = wo_pool.tile([P, D], FP32R, tag="wo")
                nc.sync.dma_start(wt[:], Wo_d.ap()[c * P : (c + 1) * P, :])
                wo_sb.append(wt)

            # 1/sums, query-major: recipN[t][:, h] = 1 / sums[h, t*128:...]
            sums_all = fsmall.tile([H, LQ], FP32R, tag="sums_all")
            nc.sync.dma_start(sums_all[:], sums_dram[:])
            recipN = [
                fsmall.tile([P, H], FP32, tag=f"recipN{t}", name=f"recipN{t}")
                for t in range(8)
            ]
            for t in range(8):
                tp = tps_ps.tile([P, H], FP32R, tag="tp_sums")
                nc.tensor.transpose(
                    tp[:], sums_all[:, t * P : (t + 1) * P], ident_r[:H, :H]
                )
                nc.vector.reciprocal(recipN[t][:], tp[:].bitcast(FP32))

            o_nat = [
                onat_pool.tile([P, D], FP32, tag="onat", name=f"onat{t}")
                for t in range(8)
            ]
            o_ln = [
                oln_pool.tile([P, D], FP32R, tag="oln", name=f"oln{t}")
                for t in range(8)
            ]
            o_lnT = [
                olnT_pool.tile([P, LQ], FP32R, tag="olnT", name=f"olnT{c}")
                for c in range(8)
            ]

            # D1: transpose O^T back to query-major, scaling by 1/sums
            for h in range(H):
                otl = otl_pool.tile([P, LQ], FP32R, tag="otl")
                nc.sync.dma_start(otl[:], ot_dram[h])
                for t in range(8):
                    tp = tp2_ps.tile([P, P], FP32R, tag="tp_d1")
                    nc.tensor.transpose(
                        tp[:], otl[:, t * P : (t + 1) * P], ident_r[:]
                    )
                    nc.vector.tensor_scalar_mul(
                        out=o_nat[t][:, h * HD : (h + 1) * HD],
                        in0=tp[:].bitcast(FP32),
                        scalar1=recipN[t][:, h : h + 1],
                    )

            # D2/D3: LayerNorm then transpose to feature-major
            if not trivial_ln:
                lng = fsmall.tile([P, D], FP32, tag="lng")
                lnb = fsmall.tile([P, D], FP32, tag="lnb")
                nc.sync.dma_start(lng[:], LNG_D.ap())
                nc.sync.dma_start(lnb[:], LNB_D.ap())
            for t in range(8):
                layernorm(fsmall, o_nat[t], o_ln[t])
                if not trivial_ln:
                    nc.vector.tensor_mul(
                        out=o_ln[t][:].bitcast(FP32),
                        in0=o_ln[t][:].bitcast(FP32),
                        in1=lng[:],
                    )
                    nc.vector.tensor_add(
                        out=o_ln[t][:],
                        in0=o_ln[t][:].bitcast(FP32),
                        in1=lnb[:],
                    )
                for c in range(8):
                    tp = tp2_ps.tile([P, P], FP32R, tag="tp_d1")
                    nc.tensor.transpose(
                        tp[:], o_ln[t][:, c * P : (c + 1) * P], ident_r[:]
                    )
                    nc.any.tensor_copy(o_lnT[c][:, t * P : (t + 1) * P], tp[:])

            # D4-7: G = O_ln @ Wo, gelu, residual, store
            for t in range(8):
                for s in range(2):
                    sl = slice(s * 512, (s + 1) * 512)
                    ps = g_ps_pool.tile([P, 512], FP32, tag="g")
                    for c in range(8):
                        nc.tensor.matmul(
                            ps[:],
                            o_lnT[c][:, t * P : (t + 1) * P],
                            wo_sb[c][:, sl],
                            start=(c == 0),
                            stop=(c == 7),
                        )
                    gel = fsmall.tile([P, 512], FP32, tag="gelu")
                    nc.scalar.activation(gel[:], ps[:], AF.Gelu)
                    outt = fsmall.tile([P, 512], FP32, tag="outsb")
                    nc.vector.tensor_add(
                        out=outt[:],
                        in0=gel[:],
                        in1=o_ln[t][:, sl].bitcast(FP32),
                    )
                    nc.sync.dma_start(OUT.ap()[t * P : (t + 1) * P, sl], outt[:])

    nc.compile()
    return nc


def _prep_host(Q, K, V, Wq, Wk, Wv, Wo, pre_g, pre_b, ln_g, ln_b):
    """Host-side preprocessing: fold pre-LN affine into weights, round fp32r."""
    pre_g = np.asarray(pre_g, np.float32)
    pre_b = np.asarray(pre_b, np.float32)
    ln_g = np.asarray(ln_g, np.float32)
    ln_b = np.asarray(ln_b, np.float32)
    Wq_eff = round_fp32r(pre_g[:, None] * np.asarray(Wq, np.float32))
    Wk_eff = round_fp32r(pre_g[:, None] * np.asarray(Wk, np.float32))
    Wv_eff = round_fp32r(np.asarray(Wv, np.float32))
    Wo_eff = round_fp32r(np.asarray(Wo, np.float32))
    has_qk_bias = bool(np.any(pre_b))
    trivial_ln = bool(np.all(ln_g == 1.0) and np.all(ln_b == 0.0))
    bq = bk = None
    if has_qk_bias:
        bq = (pre_b @ np.asarray(Wq, np.float32)).reshape(H, HD).T.copy()
        bk = (pre_b @ np.asarray(Wk, np.float32)).reshape(H, HD).T.copy()
    return Wq_eff, Wk_eff, Wv_eff, Wo_eff, has_qk_bias, trivial_ln, bq, bk, ln_g, ln_b


def kernel(Q, K, V, Wq, Wk, Wv, Wo, pre_g, pre_b, ln_g, ln_b):
    from concourse.bass_utils import run_bass_kernel_spmd

    Q = np.asarray(Q, np.float32)
    K = np.asarray(K, np.float32)
    V = np.asarray(V, np.float32)
    (Wq_e, Wk_e, Wv_e, Wo_e, has_qk_bias, trivial_ln, bq, bk, ln_g, ln_b) = _prep_host(
        Q, K, V, Wq, Wk, Wv, Wo, pre_g, pre_b, ln_g, ln_b
    )

    key = (has_qk_bias, trivial_ln)
    if key not in _PROGRAM_CACHE:
        _PROGRAM_CACHE[key] = _build_program(*key)
    nc = _PROGRAM_CACHE[key]

    ident = np.eye(P, dtype=np.float32)
    ones = np.ones((P, 1), np.float32)
    in_maps = []
    for c in range(N_CORES):
        b, half = c // 2, c % 2
        m = {
            "Qs": np.ascontiguousarray(Q[b, half * LQ : (half + 1) * LQ, :]),
            "Kf": np.ascontiguousarray(K[b]),
            "Vf": np.ascontiguousarray(V[b]),
            "Wq_r": Wq_e,
            "Wk_r": Wk_e,
            "Wv_r": Wv_e,
            "Wo_r": Wo_e,
            "ID_R": ident,
            "ID_F": ident,
            "ONES": ones,
        }
        if has_qk_bias:
            m["BQ"] = bq
            m["BK"] = bk
        if not trivial_ln:
            m["LNG_B"] = np.tile(ln_g[None, :], (P, 1))
            m["LNB_B"] = np.tile(ln_b[None, :], (P, 1))
        in_maps.append(m)

    res = run_bass_kernel_spmd(nc, in_maps, core_ids=list(range(N_CORES)))
    B = Q.shape[0]
    out = np.empty((B, 2 * LQ, D), np.float32)
    for c in range(N_CORES):
        b, half = c // 2, c % 2
        out[b, half * LQ : (half + 1) * LQ, :] = res.results[c]["OUT"]
    return out



# revision 23
# speedup vs baseline: 1.3428x; 1.3428x over previous
"""Trainium2 Bass kernel for nn_CrossAttention (B=4, L=2048, D=1024, H=8).

Sharding: 8 cores = 4 batches x 2 query-halves (data parallel over B x Lq).

Design (per core, b = c//2, half = c%2):
  - Q/K uploaded bf16; LN on DVE -> fp8e4 (+fp8 residual) feature-major
    transposes on PE; q/k projections as fp8 DoubleRow matmuls (2-term:
    (X8 + Xr8) @ W8, W8 = fp8(64*W)), PSUM evicted at scale 1/32 to fp8
    qth/kth [128hd, L]; SBUF->SBUF DMA repacks them to [64, 2, L] so the
    scores matmul S^T = k q^T runs as a DoubleRow contraction over hd.
  - exp on ScalarE (scale 1/(4*TEMP) since q,k carry 2x each) -> bf16 ex.
  - PV uses ex as the *stationary* operand: O lands query-major in PSUM
    (no transpose back), and a 1-cycle ones-matmul per (h,jc,qb) gives
    softmax sums in [128q, 1] layout for free.
  - V/Wv/Wo bf16 (the LN-amplified error-sensitive path), v = V @ Wv via
    bf16 matmuls; O evicted with 1/sums (DVE reciprocal) to bf16 o_nat;
    LN2 on DVE; Wo matmul bf16; gelu on ScalarE; residual add -> OUT fp32.
"""

import numpy as np

P = 128
D = 1024
H = 8
HD = 128
LQ = 1024  # per-core query rows
LK = 2048
N_CORES = 8
TEMP = 32.0  # sqrt(D)
EPS = 1e-5
WSCALE = 64.0  # host weight scale for fp8 Wq/Wk
QK_EVICT = 1.0 / WSCALE  # q/k psum eviction scale -> bf16 true q/k
EXP_SCALE = 1.0 / TEMP
QK_TERMS = 2  # 1: X8@W8; 2: (X8+Xr8)@W8

_PROGRAM_CACHE = {}


def _np_fp8():
    import ml_dtypes

    return ml_dtypes.float8_e4m3


def _np_bf16():
    import ml_dtypes

    return ml_dtypes.bfloat16


def _build_program(has_qk_bias: bool, trivial_ln: bool):
    import concourse.bacc as bacc
    import concourse.mybir as mybir
    import concourse.tile as tile
    from contextlib import ExitStack

    FP32 = mybir.dt.float32
    BF16 = mybir.dt.bfloat16
    FP8 = mybir.dt.float8e4
    AF = mybir.ActivationFunctionType
    SUB = mybir.AluOpType.subtract
    MULT = mybir.AluOpType.mult
    ADD = mybir.AluOpType.add
    DR = mybir.MatmulPerfMode.DoubleRow

    nc = bacc.Bacc("TRN2", target_bir_lowering=False, debug=False)

    # ---- DRAM I/O ----
    Qs = nc.dram_tensor("Qs", [LQ, D], BF16, kind="ExternalInput")
    Kf = nc.dram_tensor("Kf", [LK, D], BF16, kind="ExternalInput")
    Vf = nc.dram_tensor("Vf", [LK, D], BF16, kind="ExternalInput")
    # per-head-sliced fp8 weights: row h*128+r, col p*256+i*128+m
    # holds W64[(2p+i)*128+r, h*128+m]
    WQH = nc.dram_tensor("WQH", [D, D], FP8, kind="ExternalInput")
    WKH = nc.dram_tensor("WKH", [D, D], FP8, kind="ExternalInput")
    WVB = nc.dram_tensor("WVB", [D, D], BF16, kind="ExternalInput")
    WOB = nc.dram_tensor("WOB", [D, D], BF16, kind="ExternalInput")
    ID8 = nc.dram_tensor("ID8", [P, P], FP8, kind="ExternalInput")
    IDB = nc.dram_tensor("IDB", [P, P], BF16, kind="ExternalInput")
    ONESB = nc.dram_tensor("ONESB", [P, 1], BF16, kind="ExternalInput")
    if has_qk_bias:
        # 64 * pre_b @ W, per-head columns: [128, H]
        BQ_D = nc.dram_tensor("BQ", [P, H], FP32, kind="ExternalInput")
        BK_D = nc.dram_tensor("BK", [P, H], FP32, kind="ExternalInput")
    if not trivial_ln:
        LNG_D = nc.dram_tensor("LNG_B", [P, D], FP32, kind="ExternalInput")
        LNB_D = nc.dram_tensor("LNB_B", [P, D], FP32, kind="ExternalInput")
    OUT = nc.dram_tensor("OUT", [LQ, D], FP32, kind="ExternalOutput")

    NJK = LK // P  # 16 K/V token tiles
    NJQ = LQ // P  # 8 Q token tiles
    JBK = NJK // 4  # 4 J-blocks (512 tokens) for K
    JBQ = NJQ // 4  # 2 for Q

    with tile.TileContext(nc) as tc, ExitStack() as top:
        singles = top.enter_context(tc.tile_pool(name="singles", bufs=1))
        id8 = singles.tile([P, P], FP8)
        idb = singles.tile([P, P], BF16)
        ones_b = singles.tile([P, 1], BF16)
        eps_t = singles.tile([P, 1], FP32)
        nc.sync.dma_start(id8[:], ID8.ap())
        nc.sync.dma_start(idb[:], IDB.ap())
        nc.sync.dma_start(ones_b[:], ONESB.ap())
        nc.vector.memset(eps_t[:], EPS)
        if has_qk_bias:
            bq_sb = singles.tile([P, H], FP32)
            bk_sb = singles.tile([P, H], FP32)
            nc.sync.dma_start(bq_sb[:], BQ_D.ap())
            nc.sync.dma_start(bk_sb[:], BK_D.ap())

        # ---- persistent activation storage ----
        store = ExitStack()
        xt8K_pool = store.enter_context(
            tc.tile_pool(name="xt8K", bufs=4 * JBK * QK_TERMS)
        )
        xt8Q_pool = store.enter_context(
            tc.tile_pool(name="xt8Q", bufs=4 * JBQ * QK_TERMS)
        )
        # [J][pair] -> [128, 2(term), 2(slab), 512] fp8
        # per (J, pair): one fp8 tile per projection term, [128, 2(slab), 512]
        xt8K = [
            [[xt8K_pool.tile([P, 2, 512], FP8, tag="xtK",
                             name=f"xtK{j}_{p}_{r}")
              for r in range(QK_TERMS)]
             for p in range(4)]
            for j in range(JBK)
        ]
        xt8Q = [
            [[xt8Q_pool.tile([P, 2, 512], FP8, tag="xtQ",
                             name=f"xtQ{j}_{p}_{r}")
              for r in range(QK_TERMS)]
             for p in range(4)]
            for j in range(JBQ)
        ]
        kth_pool = store.enter_context(tc.tile_pool(name="kthp", bufs=H))
        qth_pool = store.enter_context(tc.tile_pool(name="qthp", bufs=H))
        kth = [kth_pool.tile([P, LK], BF16, tag="kth", name=f"kth{h}")
               for h in range(H)]
        qth = [qth_pool.tile([P, LQ], BF16, tag="qth", name=f"qth{h}")
               for h in range(H)]
        vnat_pool = store.enter_context(tc.tile_pool(name="vnat", bufs=NJK))
        vnat = [vnat_pool.tile([P, D], BF16, tag="vnat", name=f"vnat{t}")
                for t in range(NJK)]
        onat_pool = store.enter_context(tc.tile_pool(name="onat", bufs=NJQ))
        o_nat = [onat_pool.tile([P, D], BF16, tag="onat", name=f"onat{t}")
                 for t in range(NJQ)]

        # W window pool (per-head streaming) + proj psum + evict tmp
        wwin_pool = store.enter_context(tc.tile_pool(name="wwin", bufs=4))
        proj_ps = store.enter_context(
            tc.tile_pool(name="proj_ps", bufs=1, space="PSUM")
        )

        def emit_w_dma(h):
            """DMA the per-head W window; returns (wq_t, wk_t)."""
            wq_t = wwin_pool.tile([P, 4, 2, P], FP8, tag="wq")
            wk_t = wwin_pool.tile([P, 4, 2, P], FP8, tag="wk")
            nc.sync.dma_start(wq_t[:], WQH.ap()[h * P : (h + 1) * P, :])
            nc.sync.dma_start(wk_t[:], WKH.ap()[h * P : (h + 1) * P, :])
            return wq_t, wk_t

        def emit_proj_block(h, w_t, xt8, j, dst_tile, bias_sb):
            """One J-block (512 tokens) of head-h k/q projection + evict."""
            ps = proj_ps.tile([P, 512], FP32, tag="pj")
            n = 4 * QK_TERMS
            i = 0
            for p in range(4):
                for term in range(QK_TERMS):
                    nc.tensor.matmul(
                        ps[:],
                        w_t[:, p, :, :],
                        xt8[j][p][term][:],
                        start=(i == 0),
                        stop=(i == n - 1),
                        perf_mode=DR,
                    )
                    i += 1
            dst = dst_tile[:, j * 512 : (j + 1) * 512]
            if bias_sb is not None:
                nc.vector.tensor_scalar(
                    out=dst, in0=ps[:], scalar1=bias_sb[:, h : h + 1],
                    scalar2=QK_EVICT, op0=ADD, op1=MULT,
                )
            else:
                nc.vector.tensor_scalar_mul(out=dst, in0=ps[:], scalar1=QK_EVICT)

        def emit_head_proj_piece(h, state, piece):
            """Emit piece `piece` (0..5) of head-h projection work."""
            if piece == 0:
                state["w"] = emit_w_dma(h)
            wq_t, wk_t = state["w"]
            bq = bq_sb if has_qk_bias else None
            bk = bk_sb if has_qk_bias else None
            if piece < 4:
                emit_proj_block(h, wk_t, xt8K, piece, kth[h], bk)
            else:
                emit_proj_block(h, wq_t, xt8Q, piece - 4, qth[h], bq)

        # ================= PROLOG =================
        prolog = ExitStack()
        wv_pool = prolog.enter_context(tc.tile_pool(name="wv", bufs=8))
        xload = prolog.enter_context(tc.tile_pool(name="xload", bufs=3))
        lnt = prolog.enter_context(tc.tile_pool(name="lnt", bufs=2))
        x8p = prolog.enter_context(tc.tile_pool(name="x8p", bufs=2))
        xt8_ps = prolog.enter_context(
            tc.tile_pool(name="xt8_ps", bufs=1, space="PSUM")
        )
        vt_ps = prolog.enter_context(tc.tile_pool(name="vt_ps", bufs=1, space="PSUM"))
        vtb_pool = prolog.enter_context(tc.tile_pool(name="vtb", bufs=4))
        vp_ps = prolog.enter_context(tc.tile_pool(name="vp_ps", bufs=2, space="PSUM"))

        wv_sb = []
        for c in range(8):
            wt = wv_pool.tile([P, D], BF16, tag="wv")
            nc.sync.dma_start(wt[:], WVB.ap()[c * P : (c + 1) * P, :])
            wv_sb.append(wt)
        if not trivial_ln:
            lng = singles.tile([P, D], FP32)
            lnb = singles.tile([P, D], FP32)
            nc.sync.dma_start(lng[:], LNG_D.ap())
            nc.sync.dma_start(lnb[:], LNB_D.ap())

        def ln_stats(pool, x_tile):
            xr = x_tile[:].rearrange("p (n f) -> p n f", f=512)
            stats = pool.tile([P, 2, nc.vector.BN_STATS_DIM], FP32, tag="ln_st")
            for i in range(2):
                nc.vector.bn_stats(out=stats[:, i, :], in_=xr[:, i, :])
            mv = pool.tile([P, nc.vector.BN_AGGR_DIM], FP32, tag="ln_mv")
            nc.vector.bn_aggr(out=mv[:], in_=stats[:])
            rstd = pool.tile([P, 1], FP32, tag="ln_rs")
            nc.scalar.activation(out=rstd[:], in_=mv[:, 1:2], func=AF.Sqrt,
                                 bias=eps_t[:])
            nc.vector.reciprocal(out=rstd[:], in_=rstd[:])
            return mv, rstd

        def emit_x_tile(x_dram, t, xt8dst):
            """Load token tile t, LN (bf16), transpose; quantize at evict."""
            xt = xload.tile([P, D], BF16, tag="xl")
            nc.sync.dma_start(xt[:], x_dram.ap()[t * P : (t + 1) * P, :])
            mv, rstd = ln_stats(lnt, xt)
            j, tj = t // 4, t % 4
            xn = x8p.tile([P, D], BF16, tag="xn")
            nc.vector.tensor_scalar(
                out=xn[:], in0=xt[:], scalar1=mv[:, 0:1], scalar2=rstd[:],
                op0=SUB, op1=MULT,
            )
            for p in range(4):
                ps = xt8dst[j][p]["ps"]
                for i in range(2):
                    c = 2 * p + i
                    nc.tensor.transpose(
                        ps[:, i, tj * P : (tj + 1) * P],
                        xn[:, c * P : (c + 1) * P],
                        idb[:],
                    )
            if tj == 3:  # J-block complete: evict psum -> sbuf fp8 (+resid)
                for p in range(4):
                    ps = xt8dst[j][p]["ps"]
                    sb = xt8dst[j][p]["sb"]
                    nc.scalar.activation(sb[0][:], ps[:], AF.Copy)
                    if QK_TERMS >= 2:
                        xbf = x8p.tile([P, 2, 512], BF16, tag="xbfT")
                        nc.scalar.activation(xbf[:], ps[:], AF.Copy)
                        nc.vector.tensor_sub(
                            out=sb[1][:], in0=xbf[:], in1=sb[0][:]
                        )

        def make_xt8_ps(xt8_sb, nj):
            out = []
            for j in range(nj):
                row = []
                for p in range(4):
                    ps = xt8_ps.tile(
                        [P, 2, 512], BF16, tag=f"xps{p}",
                        name=f"xps{j}_{p}",
                    )
                    row.append({"ps": ps, "sb": xt8_sb[j][p]})
                out.append(row)
            return out

        def emit_v_tile(t):
            """V token tile t: transpose (bf16) then v-proj -> vnat[t]."""
            vt = xload.tile([P, D], BF16, tag="xl")
            nc.sync.dma_start(vt[:], Vf.ap()[t * P : (t + 1) * P, :])
            vtbs = []
            for g in range(2):  # c groups 0-3, 4-7
                ps = vt_ps.tile([P, 512], BF16, tag="vtp")
                for i in range(4):
                    c = 4 * g + i
                    nc.tensor.transpose(
                        ps[:, i * P : (i + 1) * P], vt[:, c * P : (c + 1) * P],
                        idb[:],
                    )
                vb = vtb_pool.tile([P, 512], BF16, tag="vtb")
                nc.scalar.activation(vb[:], ps[:], AF.Copy)
                vtbs.append(vb)
            for s in range(2):
                ps = vp_ps.tile([P, 512], FP32, tag="vpp")
                for c in range(8):
                    nc.tensor.matmul(
                        ps[:],
                        vtbs[c // 4][:, (c % 4) * P : (c % 4 + 1) * P],
                        wv_sb[c][:, s * 512 : (s + 1) * 512],
                        start=(c == 0),
                        stop=(c == 7),
                    )
                nc.vector.tensor_copy(vnat[t][:, s * 512 : (s + 1) * 512], ps[:])

        # interleave K-tiles (LN on DVE) with V-tiles (transpose+proj on PE)
        xt8K_ps = make_xt8_ps(xt8K, JBK)
        xt8Q_ps = make_xt8_ps(xt8Q, JBQ)
        for t in range(NJK):
            emit_x_tile(Kf, t, xt8K_ps)
            emit_v_tile(t)
        for t in range(NJQ):
            emit_x_tile(Qs, t, xt8Q_ps)

        # head-0 projection
        proj_state = {}
        for piece in range(6):
            emit_head_proj_piece(0, proj_state, piece)

        prolog.close()

        # ================= HEADS =================
        att = ExitStack()
        stp_ps = att.enter_context(tc.tile_pool(name="stp_ps", bufs=2, space="PSUM"))
        ex_pool = att.enter_context(tc.tile_pool(name="ex", bufs=3))
        o_ps_pool = att.enter_context(tc.tile_pool(name="o_ps", bufs=1, space="PSUM"))
        sm_ps_pool = att.enter_context(
            tc.tile_pool(name="sm_ps", bufs=1, space="PSUM")
        )
        rec_pool = att.enter_context(tc.tile_pool(name="rec", bufs=2))

        def st_matmul(h, jc, stp):
            for s in range(2):
                sl = slice(s * 512, (s + 1) * 512)
                nc.tensor.matmul(
                    stp[:, sl],
                    kth[h][:, jc * P : (jc + 1) * P],
                    qth[h][:, sl],
                    start=True,
                    stop=True,
                )

        for h in range(H):
            o_ps = o_ps_pool.tile([P, LQ], FP32, tag="o")
            sm_ps = sm_ps_pool.tile([P, NJQ], FP32, tag="sm")
            proj_state = {}
            stp = stp_ps.tile([P, LQ], FP32, tag="stp")
            st_matmul(h, 0, stp)
            for jc in range(NJK):
                ex = ex_pool.tile([P, LQ], BF16, tag="ex")
                nc.scalar.activation(ex[:], stp[:], AF.Exp, scale=EXP_SCALE)
                if jc + 1 < NJK:
                    stp = stp_ps.tile([P, LQ], FP32, tag="stp")
                    st_matmul(h, jc + 1, stp)
                for qb in range(NJQ):
                    exs = ex[:, qb * P : (qb + 1) * P]
                    # one accumulation group per 2KB PSUM zero-region (bank):
                    # o_ps bank = 4 qb slices; sm_ps is one region for all qb
                    nc.tensor.matmul(
                        o_ps[:, qb * P : (qb + 1) * P],
                        exs,
                        vnat[jc][:, h * HD : (h + 1) * HD],
                        start=(jc == 0 and qb % 4 == 0),
                        stop=(jc == NJK - 1 and qb % 4 == 3),
                        skip_group_check=True,
                    )
                    nc.tensor.matmul(
                        sm_ps[:, qb : qb + 1],
                        exs,
                        ones_b[:],
                        start=(jc == 0 and qb == 0),
                        stop=(jc == NJK - 1 and qb == NJQ - 1),
                        skip_group_check=True,
                    )
                if h + 1 < H and jc % 2 == 1 and jc // 2 < 6:
                    emit_head_proj_piece(h + 1, proj_state, jc // 2)
            rec = rec_pool.tile([P, NJQ], FP32, tag="rec")
            nc.vector.reciprocal(out=rec[:], in_=sm_ps[:])
            for qb in range(NJQ):
                nc.vector.tensor_scalar_mul(
                    out=o_nat[qb][:, h * HD : (h + 1) * HD],
                    in0=o_ps[:, qb * P : (qb + 1) * P],
                    scalar1=rec[:, qb : qb + 1],
                )

        att.close()

        # ================= EPILOG =================
        fin = ExitStack()
        wo_pool = fin.enter_context(tc.tile_pool(name="wo", bufs=8))
        oln_pool = fin.enter_context(tc.tile_pool(name="oln", bufs=2))
        olnT_pool = fin.enter_context(tc.tile_pool(name="olnT", bufs=2))
        fsm = fin.enter_context(tc.tile_pool(name="fsm", bufs=3))
        d3_ps = fin.enter_context(tc.tile_pool(name="d3_ps", bufs=2, space="PSUM"))
        g_ps = fin.enter_context(tc.tile_pool(name="g_ps", bufs=2, space="PSUM"))

        wo_sb = []
        for c in range(8):
            wt = wo_pool.tile([P, D], BF16, tag="wo")
            nc.sync.dma_start(wt[:], WOB.ap()[c * P : (c + 1) * P, :])
            wo_sb.append(wt)

        for t in range(NJQ):
            mv, rstd = ln_stats(fsm, o_nat[t])
            oln = oln_pool.tile([P, D], BF16, tag="oln")
            nc.vector.tensor_scalar(
                out=oln[:], in0=o_nat[t][:], scalar1=mv[:, 0:1], scalar2=rstd[:],
                op0=SUB, op1=MULT,
            )
            if not trivial_ln:
                nc.vector.tensor_mul(out=oln[:], in0=oln[:], in1=lng[:])
                nc.vector.tensor_add(out=oln[:], in0=oln[:], in1=lnb[:])
            psT = d3_ps.tile([P, D], BF16, tag="d3")
            for c in range(8):
                nc.tensor.transpose(
                    psT[:, c * P : (c + 1) * P], oln[:, c * P : (c + 1) * P],
                    idb[:],
                )
            olnTt = olnT_pool.tile([P, D], BF16, tag="olnT")
            nc.vector.tensor_copy(olnTt[:], psT[:])
            for s in range(2):
                gp = g_ps.tile([P, 512], FP32, tag="g")
                for c in range(8):
                    nc.tensor.matmul(
                        gp[:],
                        olnTt[:, c * P : (c + 1) * P],
                        wo_sb[c][:, s * 512 : (s + 1) * 512],
                        start=(c == 0),
                        stop=(c == 7),
                    )
                gel = fsm.tile([P, 512], FP32, tag="gel")
                nc.scalar.activation(gel[:], gp[:], AF.Gelu)
                outt = fsm.tile([P, 512], FP32, tag="outsb")
                nc.vector.tensor_add(
                    out=outt[:], in0=gel[:], in1=oln[:, s * 512 : (s + 1) * 512]
                )
                nc.sync.dma_start(
                    OUT.ap()[t * P : (t + 1) * P, s * 512 : (s + 1) * 512], outt[:]
                )

        fin.close()
        store.close()

    nc.compile()
    return nc


def _prep_host(Wq, Wk, Wv, Wo, pre_g, pre_b, ln_g, ln_b):
    """Fold pre-LN affine into weights; build fp8/bf16 host tensors."""
    F8 = _np_fp8()
    BF = _np_bf16()
    pre_g = np.asarray(pre_g, np.float32)
    pre_b = np.asarray(pre_b, np.float32)
    Wq_eff = pre_g[:, None] * np.asarray(Wq, np.float32)
    Wk_eff = pre_g[:, None] * np.asarray(Wk, np.float32)

    def head_slice_fp8(W64):
        w8 = W64.astype(F8)
        if QK_TERMS >= 2:
            pass  # residual is on X, not W
        # [p, i, r, h, m] -> [h, r, p, i, m] -> [1024, 1024]
        T = np.ascontiguousarray(w8).reshape(4, 2, P, H, P)
        return np.ascontiguousarray(
            T.transpose(3, 2, 0, 1, 4).reshape(D, D)
        )

    WQH = head_slice_fp8(WSCALE * Wq_eff)
    WKH = head_slice_fp8(WSCALE * Wk_eff)
    WVB = np.asarray(Wv, np.float32).astype(BF)
    WOB = np.asarray(Wo, np.float32).astype(BF)

    has_qk_bias = bool(np.any(pre_b))
    trivial_ln = bool(np.all(np.asarray(ln_g) == 1.0) and
                      np.all(np.asarray(ln_b) == 0.0))
    bq = bk = None
    if has_qk_bias:
        bq = np.ascontiguousarray(
            (WSCALE * (pre_b @ np.asarray(Wq, np.float32))).reshape(H, HD).T
        ).astype(np.float32)
        bk = np.ascontiguousarray(
            (WSCALE * (pre_b @ np.asarray(Wk, np.float32))).reshape(H, HD).T
        ).astype(np.float32)
    return WQH, WKH, WVB, WOB, has_qk_bias, trivial_ln, bq, bk


def kernel(Q, K, V, Wq, Wk, Wv, Wo, pre_g, pre_b, ln_g, ln_b):
    from concourse.bass_utils import run_bass_kernel_spmd

    F8 = _np_fp8()
    BF = _np_bf16()
    Q = np.asarray(Q, np.float32)
    K = np.asarray(K, np.float32)
    V = np.asarray(V, np.float32)
    (WQH, WKH, WVB, WOB, has_qk_bias, trivial_ln, bq, bk) = _prep_host(
        Wq, Wk, Wv, Wo, pre_g, pre_b, ln_g, ln_b
    )

    key = (has_qk_bias, trivial_ln)
    if key not in _PROGRAM_CACHE:
        _PROGRAM_CACHE[key] = _build_program(*key)
    nc = _PROGRAM_CACHE[key]

    ident8 = np.eye(P, dtype=np.float32).astype(F8)
    identb = np.eye(P, dtype=np.float32).astype(BF)
    ones_b = np.ones((P, 1), np.float32).astype(BF)
    ln_g32 = np.asarray(ln_g, np.float32)
    ln_b32 = np.asarray(ln_b, np.float32)

    in_maps = []
    for c in range(N_CORES):
        b, half = c // 2, c % 2
        m = {
            "Qs": np.ascontiguousarray(
                Q[b, half * LQ : (half + 1) * LQ, :].astype(BF)
            ),
            "Kf": np.ascontiguousarray(K[b].astype(BF)),
            "Vf": np.ascontiguousarray(V[b].astype(BF)),
            "WQH": WQH,
            "WKH": WKH,
            "WVB": WVB,
            "WOB": WOB,
            "ID8": ident8,
            "IDB": identb,
            "ONESB": ones_b,
        }
        if has_qk_bias:
            m["BQ"] = bq
            m["BK"] = bk
        if not trivial_ln:
            m["LNG_B"] = np.tile(ln_g32[None, :], (P, 1))
            m["LNB_B"] = np.tile(ln_b32[None, :], (P, 1))
        in_maps.append(m)

    res = run_bass_kernel_spmd(nc, in_maps, core_ids=list(range(N_CORES)))
    B = Q.shape[0]
    out = np.empty((B, 2 * LQ, D), np.float32)
    for c in range(N_CORES):
        b, half = c // 2, c % 2
        out[b, half * LQ : (half + 1) * LQ, :] = res.results[c]["OUT"]
    return out
